# revision 1
# baseline (speedup 1.0000x reference)
"""DSConv (deformable "snake" conv block) Trainium2 Bass kernel.

Reference math (per batch b):
  off   = conv3x3(f) + off_b -> BN(eval) -> tanh ; x_off = channels 9:18
  cum   = cumulative offsets along k from center (matrix `tri`)
  X     = h + x_off_cum (sampling x-coord; y = w + k - 4 is always integer)
  samp[b,c,w,h,k] = (0<=y<=126 and 0<=X<127) ? lerp_x(f[b,c,y,:], X) : 0
  out   = snake conv: out[b,co,w,h] = sum_{ci,k} conv_w[co,ci,0,k] samp[b,ci,w,h,k]
  out   = GroupNorm(32 groups over (4co,W,H)) -> ReLU

Sharding: 8 cores = 2 batch x 4 W-quarters (32 output rows each).

Per-core pipeline (all matmuls fp16 operands, fp32 PSUM accumulate):
  step1   Q_y[x,(k,co)] = sum_ci f[ci,y,x] wt[ci,(k,co)]          (PE)
  interp  d[h,k,x] = (x-h) + bias[h,k]  (bias = -x_off_cum, masked to BIG
          when the sample is invalid); PE-transpose d -> [x,h];
          negA = min(|d|,1)-1 = -tent(x - X)                       (DVE/GPSIMD)
  step2   out_neg[h,co] = sum_k negA_k^T @ Q_{w+k-4}[:,k-block]    (PE)
  GN      bn_stats per co, group-reduce via small matmuls, AllReduce
          across the 4 same-batch cores, relu(S*x+B) with the tent sign
          folded into S.
"""

import numpy as np
from contextlib import ExitStack

import concourse.bass as bass
import concourse.bacc as bacc
import concourse.tile as tile
import concourse.mybir as mybir
from concourse import bass_utils

F16 = mybir.dt.float16
F32 = mybir.dt.float32
ALU = mybir.AluOpType
ACTF = mybir.ActivationFunctionType

K = 9
CENTER = 4
P = 128
W = 128
H = 128
B = 2
NY = 40            # sampling row window per core: y in [w0-4, w0+36)
WC = 32            # output w-rows per core
EPS = 1e-5
EXTEND = 1.0
BIG = 30000.0
NCORES = 8


def _tri_base():
    """tri[k, kp] = coeff of x_off[k] in cumulative offset new[kp]."""
    t = np.zeros((K, K), np.float32)
    t[0, 0] = 1.0
    t[K - 1, K - 1] = 1.0
    for i in range(1, CENTER):
        for j in range(CENTER + 1, CENTER + i + 1):
            t[j, CENTER + i] = 1.0
        for j in range(CENTER - i, CENTER):
            t[j, CENTER - i] = 1.0
    return t


def build_nc():
    nc = bacc.Bacc("TRN2", target_bir_lowering=False, debug=False,
                   num_devices=NCORES)

    fs_d = nc.dram_tensor("fs", [P, NY, P], F16, kind="ExternalInput")
    fcvh_d = nc.dram_tensor("fcvh", [P, 34, 130], F16, kind="ExternalInput")
    fcvl_d = nc.dram_tensor("fcvl", [P, 34, 130], F16, kind="ExternalInput")
    wt_d = nc.dram_tensor("wt", [P, K * P], F16, kind="ExternalInput")
    owh_d = nc.dram_tensor("owh", [P, 3, 96], F16, kind="ExternalInput")
    owl_d = nc.dram_tensor("owl", [P, 3, 96], F16, kind="ExternalInput")
    bvec_d = nc.dram_tensor("bvec", [K, 1], F32, kind="ExternalInput")
    tri_d = nc.dram_tensor("tri", [K, K], F32, kind="ExternalInput")
    ski_d = nc.dram_tensor("ski", [P, P], F16, kind="ExternalInput")
    id128_d = nc.dram_tensor("id128", [P, P], F16, kind="ExternalInput")
    id9_d = nc.dram_tensor("id9", [K, K], F32, kind="ExternalInput")
    hcol_d = nc.dram_tensor("hcol", [P, 1], F32, kind="ExternalInput")
    hm127_d = nc.dram_tensor("hm127", [P, 1], F32, kind="ExternalInput")
    ybad_d = nc.dram_tensor("ybad", [P, WC, K], F16, kind="ExternalInput")
    big1_d = nc.dram_tensor("big1", [P, 1], F32, kind="ExternalInput")
    gam_d = nc.dram_tensor("gam", [P, 1], F32, kind="ExternalInput")
    bet_d = nc.dram_tensor("bet", [P, 1], F32, kind="ExternalInput")
    gmat_d = nc.dram_tensor("gmat", [P, 32], F32, kind="ExternalInput")
    gexp_d = nc.dram_tensor("gexp", [32, P], F32, kind="ExternalInput")
    out_d = nc.dram_tensor("out", [P, WC, P], F32, kind="ExternalOutput")

    cr_in = nc.dram_tensor("cr_in", [32, 2], F32, kind="Internal")
    cr_out = nc.dram_tensor("cr_out", [32, 2], F32, kind="Internal")

    with tile.TileContext(nc) as tc, ExitStack() as ctx:
        const = ctx.enter_context(tc.tile_pool(name="const", bufs=1))
        mid = ctx.enter_context(tc.tile_pool(name="mid", bufs=1))

        def load(name, dram, shape, dtype):
            t = const.tile(shape, dtype, name=name + "_sb")
            nc.sync.dma_start(out=t, in_=dram.ap())
            return t

        fs = load("fs", fs_d, [P, NY, P], F16)
        fcvh = load("fcvh", fcvh_d, [P, 34, 130], F16)
        fcvl = load("fcvl", fcvl_d, [P, 34, 130], F16)
        wt = load("wt", wt_d, [P, K * P], F16)
        owh = load("owh", owh_d, [P, 3, 96], F16)
        owl = load("owl", owl_d, [P, 3, 96], F16)
        bvec = load("bvec", bvec_d, [K, 1], F32)
        tri_t = load("tri", tri_d, [K, K], F32)
        ski = load("ski", ski_d, [P, P], F16)
        id128 = load("id128", id128_d, [P, P], F16)
        id9 = load("id9", id9_d, [K, K], F32)
        hcol = load("hcol", hcol_d, [P, 1], F32)
        hm127 = load("hm127", hm127_d, [P, 1], F32)
        ybad = load("ybad", ybad_d, [P, WC, K], F16)
        big1 = load("big1", big1_d, [P, 1], F32)
        gam = load("gam", gam_d, [P, 1], F32)
        bet = load("bet", bet_d, [P, 1], F32)
        gmat = load("gmat", gmat_d, [P, 32], F32)
        gexp = load("gexp", gexp_d, [32, P], F32)

        xoff = mid.tile([K, WC * P], F32)
        braw = mid.tile([K, WC * P], F32)
        biasF = mid.tile([P, WC, K], F32)
        outC = mid.tile([P, WC, P], F16)
        final = mid.tile([P, WC, P], F32)

        # ---- offset branch ----
        with tc.tile_pool(name="psB", bufs=2, space="PSUM") as psB, \
             tc.tile_pool(name="psD", bufs=2, space="PSUM") as psD:
            x9 = mid.tile([K, WC, P], F32)
            g0t = mid.tile([K, 3, 130], F32)
            g1t = mid.tile([K, 3, P], F32)
            wchunks = [(c3, min(3, WC - c3)) for c3 in range(0, WC, 3)]
            for c3, nw in wchunks:
                ps = psB.tile([96, 390], F32, tag="off", name="ps_off")
                pss = ps.rearrange("p (a b) -> p a b", a=3)[:, :nw, :]
                first, last = True, False
                for dw in range(3):
                    last = (dw == 2)
                    rh = fcvh[:, c3 + dw: c3 + dw + nw, :]
                    rl = fcvl[:, c3 + dw: c3 + dw + nw, :]
                    nc.tensor.matmul(pss, lhsT=owh[:, dw, :], rhs=rh,
                                     start=first, stop=False)
                    first = False
                    nc.tensor.matmul(pss, lhsT=owl[:, dw, :], rhs=rh,
                                     start=False, stop=False)
                    nc.tensor.matmul(pss, lhsT=owh[:, dw, :], rhs=rl,
                                     start=False, stop=last)
                # dh-recombine: groups at psum partitions 0/32/64, h-shifts 0/1/2
                nc.vector.tensor_copy(out=g0t[:, :nw, :], in_=pss[0:K, :, :])
                nc.vector.tensor_tensor(
                    out=g1t[:, :nw, :], in0=pss[32:32 + K, :, 1:129],
                    in1=g0t[:, :nw, 0:128], op=ALU.add)
                nc.vector.tensor_tensor(
                    out=x9[:, c3: c3 + nw, :], in0=pss[64:64 + K, :, 2:130],
                    in1=g1t[:, :nw, :], op=ALU.add)
            for c8 in range(8):
                nc.scalar.activation(
                    out=xoff[:, c8 * 512:(c8 + 1) * 512],
                    in_=x9.rearrange("p a b -> p (a b)")[:, c8 * 512:(c8 + 1) * 512],
                    func=ACTF.Tanh, bias=bvec, scale=1.0)
            for c8 in range(8):
                ps = psB.tile([K, 512], F32, tag="off", name="ps_cum")
                nc.tensor.matmul(ps, lhsT=tri_t,
                                 rhs=xoff[:, c8 * 512:(c8 + 1) * 512],
                                 start=True, stop=True)
                nc.vector.tensor_copy(out=braw[:, c8 * 512:(c8 + 1) * 512],
                                      in_=ps)
            for wl in range(WC):
                pst = psD.tile([P, K], F32, tag="xt", name="ps_xt")
                nc.tensor.transpose(pst, braw[:, wl * 128:(wl + 1) * 128], id9)
                nc.vector.tensor_copy(out=biasF[:, wl, :], in_=pst)
            # invalid sample (X<0 i.e. braw>h, or X>=127 i.e. braw<=h-127) -> BIG
            i1 = mid.tile([P, WC, K], mybir.dt.uint8)
            i2 = mid.tile([P, WC, K], mybir.dt.uint8)
            nc.vector.tensor_scalar(out=i1, in0=biasF, scalar1=hcol,
                                    scalar2=None, op0=ALU.is_gt)
            nc.vector.tensor_scalar(out=i2, in0=biasF, scalar1=hm127,
                                    scalar2=None, op0=ALU.is_le)
            nc.vector.tensor_tensor(out=i1, in0=i1, in1=i2, op=ALU.logical_or)
            nc.vector.copy_predicated(
                out=biasF.rearrange("p w k -> p (w k)"),
                mask=i1.rearrange("p w k -> p (w k)"),
                data=big1.to_broadcast([P, WC * K]))
            nc.vector.tensor_tensor(out=biasF, in0=biasF, in1=ybad, op=ALU.add)

        # ---- step1 + interp + step2 ----
        with tc.tile_pool(name="qwin", bufs=14) as qpool, \
             tc.tile_pool(name="dbuf", bufs=3) as dpool, \
             tc.tile_pool(name="psQ", bufs=2, space="PSUM") as psQ, \
             tc.tile_pool(name="psT", bufs=2, space="PSUM") as psT, \
             tc.tile_pool(name="psO", bufs=1, space="PSUM") as psO:
            qtiles = {}

            def q_row(yl):
                q = qpool.tile([P, K, P], F16, tag="q", name="q_sb")
                qf = q.rearrange("p k c -> p (k c)")
                for j in range(3):
                    ps = psQ.tile([P, 384], F32, tag="q3", name="ps_q")
                    nc.tensor.matmul(ps, lhsT=fs[:, yl, :],
                                     rhs=wt[:, j * 384:(j + 1) * 384],
                                     start=True, stop=True)
                    if j == 0:
                        nc.vector.tensor_copy(
                            out=qf[:, j * 384:(j + 1) * 384], in_=ps)
                    else:
                        nc.scalar.copy(
                            out=qf[:, j * 384:(j + 1) * 384], in_=ps)
                qtiles[yl] = q

            for yl in range(K - 1):
                q_row(yl)
            for wl in range(WC):
                q_row(wl + K - 1)
                d16 = dpool.tile([P, K, P], F16, tag="d", name="d16")
                nc.gpsimd.tensor_tensor(
                    out=d16,
                    in0=ski.unsqueeze(1).to_broadcast([P, K, P]),
                    in1=biasF[:, wl, :].unsqueeze(2).to_broadcast([P, K, P]),
                    op=ALU.add)
                u0 = dpool.tile([P, K, P], F16, tag="u", name="u16")
                nc.scalar.activation(out=u0, in_=d16, func=ACTF.Abs)
                pst = psT.tile([P, K, P], F16, tag="at", name="ps_at")
                for k in range(K):
                    nc.tensor.matmul(pst[:, k, :], lhsT=u0[:, k, :],
                                     rhs=id128, is_transpose=True,
                                     start=True, stop=True,
                                     skip_group_check=True)
                negA = dpool.tile([P, K, P], F16, tag="na", name="negA")
                nc.vector.tensor_scalar(out=negA, in0=pst, scalar1=1.0,
                                        scalar2=1.0, op0=ALU.min,
                                        op1=ALU.subtract)
                po = psO.tile([P, P], F32, tag="oacc", name="ps_o")
                for k in range(K):
                    nc.tensor.matmul(po, lhsT=negA[:, k, :],
                                     rhs=qtiles[wl + k][:, k, :],
                                     start=(k == 0), stop=(k == K - 1))
                ocp = dpool.tile([P, P], F16, tag="ocp", name="ocp")
                if wl % 2 == 0:
                    nc.vector.tensor_copy(out=ocp, in_=po)
                else:
                    nc.scalar.copy(out=ocp, in_=po)
                pw = psT.tile([P, P], F16, tag="ot", name="ps_ot", bufs=1)
                nc.tensor.transpose(pw, ocp, id128)
                if wl % 2 == 0:
                    nc.scalar.copy(out=outC[:, wl, :], in_=pw)
                else:
                    nc.vector.tensor_copy(out=outC[:, wl, :], in_=pw)
                del qtiles[wl]

            # ---- GroupNorm ----
            outCf = outC.rearrange("p w h -> p (w h)")
            stats = mid.tile([P, 8, 6], F32)
            for c8 in range(8):
                nc.vector.bn_stats(out=stats[:, c8, :],
                                   in_=outCf[:, c8 * 512:(c8 + 1) * 512])
            mv = mid.tile([P, 2], F32)
            nc.vector.bn_aggr(out=mv, in_=stats)
            st2 = mid.tile([P, 2], F32)
            nc.vector.tensor_copy(out=st2[:, 0:1], in_=mv[:, 0:1])
            sq = mid.tile([P, 1], F32)
            nc.vector.tensor_tensor(out=sq, in0=mv[:, 0:1], in1=mv[:, 0:1],
                                    op=ALU.mult)
            nc.vector.tensor_tensor(out=st2[:, 1:2], in0=mv[:, 1:2], in1=sq,
                                    op=ALU.add)
            pg = psO.tile([32, 2], F32, tag="oacc", name="ps_g")
            nc.tensor.matmul(pg, lhsT=gmat, rhs=st2, start=True, stop=True)
            g2 = mid.tile([32, 2], F32)
            nc.vector.tensor_copy(out=g2, in_=pg)
            nc.sync.dma_start(out=cr_in.ap(), in_=g2)
            nc.gpsimd.collective_compute(
                kind="AllReduce", op=ALU.add,
                replica_groups=[[0, 1, 2, 3], [4, 5, 6, 7]],
                ins=[cr_in.ap()], outs=[cr_out.ap()])
            nc.sync.dma_start(out=g2, in_=cr_out.ap())
            nc.vector.tensor_scalar(out=g2, in0=g2, scalar1=0.25, scalar2=None,
                                    op0=ALU.mult)
            m2 = mid.tile([32, 1], F32)
            nc.vector.tensor_tensor(out=m2, in0=g2[:, 0:1], in1=g2[:, 0:1],
                                    op=ALU.mult)
            vg = mid.tile([32, 1], F32)
            nc.vector.tensor_tensor(out=vg, in0=g2[:, 1:2], in1=m2,
                                    op=ALU.subtract)
            nc.vector.tensor_scalar(out=vg, in0=vg, scalar1=EPS, scalar2=None,
                                    op0=ALU.add)
            nc.scalar.sqrt(out=vg, in_=vg)
            nc.vector.reciprocal(out=vg, in_=vg)
            g3 = mid.tile([32, 2], F32)
            nc.vector.tensor_copy(out=g3[:, 0:1], in_=g2[:, 0:1])
            nc.vector.tensor_copy(out=g3[:, 1:2], in_=vg)
            pe2 = psO.tile([P, 2], F32, tag="oacc", name="ps_e2")
            nc.tensor.matmul(pe2, lhsT=gexp, rhs=g3, start=True, stop=True)
            ec = mid.tile([P, 2], F32)
            nc.vector.tensor_copy(out=ec, in_=pe2)
            t1 = mid.tile([P, 1], F32)
            nc.vector.tensor_tensor(out=t1, in0=ec[:, 1:2], in1=gam,
                                    op=ALU.mult)
            Sv = mid.tile([P, 1], F32)
            nc.vector.tensor_scalar(out=Sv, in0=t1, scalar1=-1.0, scalar2=None,
                                    op0=ALU.mult)
            t2 = mid.tile([P, 1], F32)
            nc.vector.tensor_tensor(out=t2, in0=ec[:, 0:1], in1=t1,
                                    op=ALU.mult)
            Bv = mid.tile([P, 1], F32)
            nc.vector.tensor_tensor(out=Bv, in0=t2, in1=bet, op=ALU.add)
            for c4 in range(0, WC, 4):
                nc.scalar.activation(out=final[:, c4:c4 + 4, :],
                                     in_=outC[:, c4:c4 + 4, :],
                                     func=ACTF.Relu, bias=Bv, scale=Sv)
            nc.sync.dma_start(out=out_d.ap(), in_=final)

    nc.compile()
    return nc


_TRI = _tri_base()


def prep_shared(off_w, off_b, bn_gamma, bn_beta, bn_mean, bn_var, conv_w,
                gn_gamma, gn_beta):
    s36 = (np.asarray(bn_gamma, np.float32)
           / np.sqrt(np.asarray(bn_var, np.float32) + EPS))
    s = s36[K:2 * K]
    bvec = ((np.asarray(off_b, np.float32)[K:2 * K]
             - np.asarray(bn_mean, np.float32)[K:2 * K]) * s
            + np.asarray(bn_beta, np.float32)[K:2 * K]
            ).reshape(K, 1).astype(np.float32)

    owf = np.asarray(off_w, np.float32)[K:2 * K]          # [k, ci, dw, dh]
    oww = np.zeros((P, 3, 96), np.float32)                # [ci, dw, (dh-group, k)]
    for dw in range(3):
        for dh in range(3):
            oww[:, dw, dh * 32: dh * 32 + K] = (owf[:, :, dw, dh] * s[:, None]).T
    owh = oww.astype(np.float16)
    owl = (oww - owh.astype(np.float32)).astype(np.float16)

    wtf = np.asarray(conv_w, np.float32)[:, :, 0, :]      # [co, ci, k]
    wt = np.ascontiguousarray(
        np.transpose(wtf, (1, 2, 0)).reshape(P, K * P)).astype(np.float16)

    hx = np.arange(P, dtype=np.float32)
    return dict(
        wt=wt, owh=owh, owl=owl, bvec=bvec,
        tri=(-EXTEND * _TRI).astype(np.float32),
        ski=(hx[None, :] - hx[:, None]).astype(np.float16),
        id128=np.eye(P, dtype=np.float16),
        id9=np.eye(K, dtype=np.float32),
        hcol=hx.reshape(P, 1).astype(np.float32),
        hm127=(hx - 127.0).reshape(P, 1).astype(np.float32),
        big1=np.full((P, 1), BIG, np.float32),
        gam=np.asarray(gn_gamma, np.float32).reshape(P, 1),
        bet=np.asarray(gn_beta, np.float32).reshape(P, 1),
        gmat=np.array([[0.25 if co // 4 == g else 0.0 for g in range(32)]
                       for co in range(P)], np.float32),
        gexp=np.array([[1.0 if co // 4 == g else 0.0 for co in range(P)]
                       for g in range(32)], np.float32),
    )


def prep_core(f, b, w0):
    fb = np.asarray(f, np.float32)[b]
    fs = np.zeros((P, NY, P), np.float16)
    lo = max(0, w0 - 4)
    hi = min(W, w0 + 36)
    fs[:, lo - (w0 - 4): hi - (w0 - 4), :] = fb[:, lo:hi, :].astype(np.float16)
    fpad = np.pad(fb, ((0, 0), (1, 1), (1, 1)))
    fcv = np.ascontiguousarray(fpad[:, w0:w0 + 34, :])
    fcvh = fcv.astype(np.float16)
    fcvl = (fcv - fcvh.astype(np.float32)).astype(np.float16)
    ybad = np.zeros((P, WC, K), np.float16)
    for wl in range(WC):
        for k in range(K):
            y = w0 + wl + k - 4
            if not (0 <= y <= 126):
                ybad[:, wl, k] = BIG
    return dict(fs=fs, fcvh=fcvh, fcvl=fcvl, ybad=ybad)


_NC_CACHE = {}


def get_nc():
    if "nc" not in _NC_CACHE:
        _NC_CACHE["nc"] = build_nc()
    return _NC_CACHE["nc"]


def make_in_maps(f, off_w, off_b, bn_gamma, bn_beta, bn_mean, bn_var,
                 conv_w, conv_b, gn_gamma, gn_beta):
    consts = prep_shared(off_w, off_b, bn_gamma, bn_beta, bn_mean, bn_var,
                         conv_w, gn_gamma, gn_beta)
    in_maps = []
    for c in range(NCORES):
        b, q = c // 4, c % 4
        m = dict(consts)
        m.update(prep_core(f, b, q * WC))
        in_maps.append(m)
    return in_maps


def assemble(results):
    out = np.zeros((B, P, W, H), np.float32)
    for c in range(NCORES):
        b, q = c // 4, c % 4
        out[b, :, q * WC:(q + 1) * WC, :] = results[c]["out"]
    return out


def kernel(f, off_w, off_b, bn_gamma, bn_beta, bn_mean, bn_var,
           conv_w, conv_b, gn_gamma, gn_beta, **run_kwargs):
    nc = get_nc()
    in_maps = make_in_maps(f, off_w, off_b, bn_gamma, bn_beta, bn_mean,
                           bn_var, conv_w, conv_b, gn_gamma, gn_beta)
    last_exc = None
    for _attempt in range(3):
        try:
            res = bass_utils.run_bass_kernel_spmd(
                nc, in_maps, core_ids=list(range(NCORES)), **run_kwargs)
            break
        except Exception as e:  # transient tunnel/device hiccups
            last_exc = e
    else:
        raise last_exc
    out = assemble(res.results)
    kernel.last_result = res
    return out

